# revision 27
# baseline (speedup 1.0000x reference)
"""ChebyASPIRE spectral filter on 8 TRN2 NeuronCores.

Symmetric Gram build + spectral deflation + low-degree Chebyshev refit.
Phase 1 computes only ~4.5 of the 8 (512x512) blocks of each core's
column of Z = X^T X (peer-oriented, lhsT = own resident X[:, ib]) and
exchanges the rest via ONE full-group AllToAll (a single extra
replica-group config -- many distinct configs crash the runtime).
Per-core slot/placement selection uses cond-predicated DMAs driven by
0/1 flags from an input tensor.  Deflation (- lam1 u u^T) folds into
the PSUM->SBUF copy via host outer-product tiles; phase 2 runs deg(q)
(~3) rescaled Chebyshev steps with fp8 AllGathers and ends with the
exact rank-1 patch beta*u*w^T.
"""
import sys

sys.path.insert(0, "/opt/trn_rl_repo")

import numpy as np

M, N, B = 8192, 4096, 256
NC = 8
CB = N // NC
KT1 = M // 128
KP1 = KT1 // 2
KT4 = KT1 // 2
KP4 = KT4 // 2
KT2 = N // 128
MS2 = CB // 128
NH = 2
BH = B // NH
XSCALE = 32.0
ZDESCALE = 1.0 / (XSCALE * XSCALE)
RCH = 8
NBLK = 5              # b1, b2, b3, b4(half-k), b0(diag)

_BUILD_CACHE = {}


def _build(key):
    from concourse import bacc, tile, mybir

    tm, th, deg = key[0], key[1], int(key[2])
    c = key[3:]
    assert len(c) == deg + 1
    f8 = mybir.dt.float8e4
    f16 = mybir.dt.float16
    f32 = mybir.dt.float32
    i32 = mybir.dt.int32
    mult = mybir.AluOpType.mult
    add = mybir.AluOpType.add
    sub = mybir.AluOpType.subtract
    DR = mybir.MatmulPerfMode.DoubleRow

    nc = bacc.Bacc("TRN2", target_bir_lowering=False, debug=False,
                   num_devices=NC)
    XR = nc.dram_tensor("XR8", [128 * KT1, CB], f8, kind="ExternalInput")
    XS = nc.dram_tensor("XS8", [3 * 128 * KT1, CB], f8,
                        kind="ExternalInput")
    XS4 = nc.dram_tensor("XS84", [128 * KT4, CB], f8, kind="ExternalInput")
    XLH = nc.dram_tensor("XLH8", [128 * KT4, CB], f8, kind="ExternalInput")
    OU = nc.dram_tensor("OU16", [NBLK * MS2 * 128, CB], f16,
                        kind="ExternalInput")
    CND = nc.dram_tensor("CND32", [1, 80], i32, kind="ExternalInput")
    VL = nc.dram_tensor("VL8", [NH * 128 * KT2, BH], f8,
                        kind="ExternalInput")
    Vb = nc.dram_tensor("Vblk32", [CB, B], f32, kind="ExternalInput")
    U16 = nc.dram_tensor("U16", [1, CB], f16, kind="ExternalInput")
    BW = nc.dram_tensor("BW16", [1, B], f16, kind="ExternalInput")
    acc_out = nc.dram_tensor("acc_out", [CB, B], f32, kind="ExternalOutput")

    XR3 = XR[:, :].rearrange("(p kk) cb -> p kk cb", p=128)
    XS3 = XS[:, :].rearrange("(r cc p kk) cb -> r cc p kk cb",
                         r=3, cc=KT1 // RCH, p=128)
    XS43 = XS4[:, :].rearrange("(cc p kk) cb -> cc p kk cb",
                           cc=KT4 // RCH, p=128)
    XLH3 = XLH[:, :].rearrange("(p kk) cb -> p kk cb", p=128)
    OU3 = OU[:, :].rearrange("(bm p) cb -> p bm cb", p=128)

    with tile.TileContext(nc) as tc:
        with (
            tc.tile_pool(name="persist", bufs=1) as persist,
            tc.tile_pool(name="xsp", bufs=8) as xsp,
            tc.tile_pool(name="oup", bufs=4) as oup,
            tc.tile_pool(name="blkp", bufs=3) as blkp,
            tc.tile_pool(name="rcvp", bufs=1) as rcvp,
            tc.tile_pool(name="blkTp", bufs=2) as blkTp,
            tc.tile_pool(name="rhsp", bufs=2) as rhsp,
            tc.tile_pool(name="dve", bufs=4) as dvep,
            tc.tile_pool(name="stagep", bufs=2) as stagep,
            tc.tile_pool(name="ps1", bufs=2, space="PSUM") as ps1,
            tc.tile_pool(name="ps2", bufs=6, space="PSUM") as ps2p,
            tc.tile_pool(name="dram", bufs=1, space="DRAM") as dram,
        ):
            rhs_res = [persist.tile([128, RCH, CB], f8, name=f"rhs_res{cc}")
                       for cc in range(KT1 // RCH)]
            for cc in range(KT1 // RCH):
                nc.sync.dma_start(rhs_res[cc][:],
                                  XR3[:, cc * RCH:(cc + 1) * RCH, :])
            xlh = persist.tile([128, KT4, CB], f8, name="xlh")
            nc.sync.dma_start(xlh[:], XLH3[:, :, :])
            cnd = persist.tile([1, 80], i32, name="cnd")
            nc.sync.dma_start(cnd[:], CND[:, :])

            tstate = [[persist.tile([128, MS2, BH], f32, name=f"tst{h}_{i}")
                       for i in range(3)] for h in range(NH)]
            acc = [persist.tile([128, MS2, BH], f32, name=f"acc{h}")
                   for h in range(NH)]
            zero = persist.tile([128, BH], f32, name="zero")
            nc.any.memset(zero[:], 0.0)
            Vb3 = Vb[:, :].rearrange("(ms p) b -> p ms b", p=128)
            for h in range(NH):
                nc.sync.dma_start(tstate[h][0][:],
                                  Vb3[:, :, h * BH:(h + 1) * BH])
            u16 = persist.tile([1, CB], f16, name="u16")
            bw = persist.tile([1, B], f16, name="bw")
            nc.sync.dma_start(u16[:], U16[:, :])
            nc.sync.dma_start(bw[:], BW[:, :])

            agin_w = dram.tile([128, MS2 * BH], f8, name="agin_w")
            agout_w = dram.tile([NC * 128, MS2 * BH], f8,
                                addr_space="Shared", name="agout_w")
            nc.gpsimd.collective_compute(
                "AllGather", mybir.AluOpType.bypass,
                replica_groups=[list(range(NC))],
                ins=[agin_w[:]], outs=[agout_w[:]])

            zkbig = persist.tile([128, KT2, CB], f16, name="zkbig")
            oubig = persist.tile([128, NBLK * MS2, CB], f16, name="oubig")
            nc.sync.dma_start(oubig[:], OU3[:, :, :])

            def pred_dma(engine, dst, src, flag_idx, tag):
                r = engine.alloc_register(f"c_{tag}")
                engine.reg_load(r, cnd[0:1, flag_idx:flag_idx + 1])
                rv = engine.snap(r, donate=True, min_val=0, max_val=1)
                engine.dma_start(dst, src, cond=rv)

            # cnd layout: [diag(8) | self_r1(8) | self_r2 | self_r3 |
            #              recv_r1(8) | recv_r2 | recv_r3 | b4(8) | pad]
            C_DIAG = 0
            C_SELF = [8, 16, 24]      # also the A2A send slot for b_r
            C_RECV = [32, 40, 48]
            C_B4 = 56                 # send slot, recv slot, placement

            # full-group AllToAll: slot s = 512KB block for core s
            a2a_in = dram.tile([NC * 128, MS2 * CB], f16, name="a2a_in")
            a2a_out = dram.tile([NC * 128, MS2 * CB], f16, name="a2a_out")

            def slot(t, s):
                return (t[s * 128:(s + 1) * 128, :]
                        .rearrange("p (mc cb) -> p mc cb", cb=CB))

            blk4 = None
            for b in (3, 0, 1, 2, 4):
                nkp = KP4 if b == 3 else KP1
                blk = blkp.tile([128, MS2, CB], f16, name="blk")
                if b < 3:
                    xs_ch = [xsp.tile([128, RCH, CB], f8, name="xs")
                             for cc in range(KT1 // RCH)]
                    xq = nc.scalar if b == 0 else nc.sync
                    for cc in range(KT1 // RCH):
                        xq.dma_start(xs_ch[cc][:],
                                     XS3[b, cc, :, :, :])
                elif b == 3:
                    xs_ch = [xsp.tile([128, RCH, CB], f8, name="xs")
                             for cc in range(KT4 // RCH)]
                    for cc in range(KT4 // RCH):
                        nc.scalar.dma_start(xs_ch[cc][:],
                                            XS43[cc, :, :, :])
                else:
                    xs_ch = None

                for mc in range(MS2):
                    zps = ps1.tile([128, CB], f32, name="zps")
                    for kp in range(nkp):
                        kk = 2 * kp
                        if b == 3:
                            lhsT = xlh[:, kk:kk + 2,
                                       mc * 128:(mc + 1) * 128]
                        else:
                            lhsT = rhs_res[kk // RCH][
                                :, kk % RCH:kk % RCH + 2,
                                mc * 128:(mc + 1) * 128]
                        if b == 4:
                            rhs = rhs_res[kk // RCH][
                                :, kk % RCH:kk % RCH + 2, :]
                        else:
                            rhs = xs_ch[kk // RCH][
                                :, kk % RCH:kk % RCH + 2, :]
                        nc.tensor.matmul(zps[:], lhsT, rhs,
                                         start=(kp == 0),
                                         stop=(kp == nkp - 1),
                                         perf_mode=DR)
                    nc.vector.tensor_scalar_mul(blk[:, mc, :], zps[:],
                                                ZDESCALE)
                    nc.vector.tensor_sub(
                        blk[:, mc, :], blk[:, mc, :],
                        oubig[:, b * MS2 + mc, :])

                if b < 4:
                    # ship peer-oriented block: slot (i + r) % 8
                    cbase = C_SELF[b] if b < 3 else C_B4
                    for s in range(8):
                        pred_dma(nc.sync, slot(a2a_in, s)[:], blk[:],
                                 cbase + s, f"snd{b}s{s}")

                if b < 3:
                    # own copy, transposed -> zk rows (i+r)
                    blkT = blkTp.tile([128, MS2, CB], f16, name="blkT")
                    for rc in range(MS2):
                        for cc in range(MS2):
                            nc.sync.dma_start_transpose(
                                blkT[:, cc, rc * 128:(rc + 1) * 128],
                                blk[:, rc, cc * 128:(cc + 1) * 128])
                    st = dram.tile([128, MS2 * CB], f16, name=f"st{b}")
                    st3 = st[:, :].rearrange("p (mc cb) -> p mc cb", cb=CB)
                    nc.sync.dma_start(st3[:], blkT[:])
                    for g in range(8):
                        pred_dma(nc.sync, zkbig[:, 4 * g:4 * g + 4, :],
                                 st3[:], C_SELF[b] + g, f"s{b}g{g}")

                if b == 3:
                    # own half, transposed (kept until after the A2A)
                    blk4 = blk
                    blkT4 = persist.tile([128, MS2, CB], f16, name="blkT4")
                    for rc in range(MS2):
                        for cc in range(MS2):
                            nc.sync.dma_start_transpose(
                                blkT4[:, cc, rc * 128:(rc + 1) * 128],
                                blk[:, rc, cc * 128:(cc + 1) * 128])

                if b == 2:
                    # last exchange block staged -> one AllToAll
                    nc.gpsimd.collective_compute(
                        "AllToAll", mybir.AluOpType.bypass,
                        replica_groups=[list(range(NC))],
                        ins=[a2a_in[:]], outs=[a2a_out[:]])

                if b == 4:
                    std = dram.tile([128, MS2 * CB], f16, name="std")
                    std3 = std[:, :].rearrange("p (mc cb) -> p mc cb",
                                               cb=CB)
                    nc.sync.dma_start(std3[:], blk[:])
                    for g in range(8):
                        pred_dma(nc.sync, zkbig[:, 4 * g:4 * g + 4, :],
                                 std3[:], C_DIAG + g, f"dg{g}")

            # ---- consume the AllToAll (after all local compute) ----
            recv4 = rcvp.tile([128, MS2, CB], f16, name="recv4")
            for s in range(8):
                pred_dma(nc.scalar, recv4[:], slot(a2a_out, s)[:],
                         C_B4 + s, f"b4r{s}")
            nc.vector.tensor_add(blkT4[:], blkT4[:], recv4[:])
            st4 = dram.tile([128, MS2 * CB], f16, name="st4")
            st43 = st4[:, :].rearrange("p (mc cb) -> p mc cb", cb=CB)
            nc.sync.dma_start(st43[:], blkT4[:])
            for g in range(8):
                pred_dma(nc.sync, zkbig[:, 4 * g:4 * g + 4, :],
                         st43[:], C_B4 + g, f"b4g{g}")
            for rr in range(3):
                for g in range(8):
                    pred_dma(nc.scalar, zkbig[:, 4 * g:4 * g + 4, :],
                             slot(a2a_out, g)[:], C_RECV[rr] + g,
                             f"r{rr}g{g}")

            # ---------------- phase 2 ----------------
            agoutP = [[None] * NH for _ in range(deg)]
            out3 = acc_out[:, :].rearrange("(ms p) b -> p ms b", p=128)

            for s in range(1, deg + 1):
                for h in range(NH):
                    rh = rhsp.tile([128, NC, MS2, BH], f8, name=f"rh{h}")
                    if s == 1:
                        src = (VL[h * 128 * KT2:(h + 1) * 128 * KT2, :]
                               .rearrange("(p r ms) b -> p r ms b",
                                          p=128, r=NC))
                        nc.scalar.dma_start(rh[:], src[:])
                    else:
                        src = (agoutP[s - 2][h][:, :]
                               .rearrange("(r p) (ms b) -> p r ms b",
                                          p=128, b=BH))
                        nc.scalar.dma_start(rh[:], src[:])

                    Tc = tstate[h][(s - 1) % 3]
                    Tp = tstate[h][(s - 2) % 3] if s >= 2 else None
                    Tn = tstate[h][s % 3]
                    ach = acc[h]
                    if s < deg:
                        stage = stagep.tile([128, MS2, BH], f8,
                                            name=f"stage{h}")
                        agins = dram.tile([128, MS2 * BH], f8,
                                          name=f"agin{s}_{h}")
                        agin3 = agins[:, :].rearrange("p (ms b) -> p ms b",
                                                      b=BH)

                    for ms in range(MS2):
                        wps = ps2p.tile([128, BH], f32, name="wps")
                        for kk in range(KT2):
                            nc.tensor.matmul(
                                wps[:],
                                zkbig[:, kk, ms * 128:(ms + 1) * 128],
                                rh[:, kk // MS2, kk % MS2, :],
                                start=(kk == 0), stop=(kk == KT2 - 1))

                        u = dvep.tile([128, BH], f32, name="u")
                        nc.vector.scalar_tensor_tensor(
                            u[:], Tc[:, ms, :], -tm, wps[:],
                            op0=mult, op1=add)
                        if s == 1:
                            nc.vector.scalar_tensor_tensor(
                                Tn[:, ms, :], u[:], 1.0 / th, zero[:],
                                op0=mult, op1=sub)
                            nc.vector.tensor_scalar_mul(
                                ach[:, ms, :], Tc[:, ms, :], c[0])
                            nc.vector.scalar_tensor_tensor(
                                ach[:, ms, :], Tn[:, ms, :], c[1],
                                ach[:, ms, :], op0=mult, op1=add)
                        else:
                            nc.vector.scalar_tensor_tensor(
                                Tn[:, ms, :], u[:], 2.0 / th, Tp[:, ms, :],
                                op0=mult, op1=sub)
                        if s < deg:
                            nc.vector.tensor_copy(stage[:, ms, :],
                                                  Tn[:, ms, :])
                        if s > 1:
                            nc.vector.scalar_tensor_tensor(
                                ach[:, ms, :], Tn[:, ms, :], c[s],
                                ach[:, ms, :], op0=mult, op1=add)

                    if s < deg:
                        nc.sync.dma_start(agin3[:], stage[:])
                        agoutP[s - 1][h] = dram.tile(
                            [NC * 128, MS2 * BH], f8, addr_space="Shared",
                            name=f"agoutP{s}_{h}")
                        nc.gpsimd.collective_compute(
                            "AllGather", mybir.AluOpType.bypass,
                            replica_groups=[list(range(NC))],
                            ins=[agins[:]], outs=[agoutP[s - 1][h][:]])
                    else:
                        for ms in range(MS2):
                            pr1 = ps2p.tile([128, BH], f32, name="wps")
                            nc.tensor.matmul(
                                pr1[:],
                                u16[:, ms * 128:(ms + 1) * 128],
                                bw[:, h * BH:(h + 1) * BH],
                                start=True, stop=True)
                            nc.vector.scalar_tensor_tensor(
                                acc[h][:, ms, :], pr1[:], 1.0,
                                acc[h][:, ms, :], op0=mult, op1=add)
                        nc.sync.dma_start(out3[:, :, h * BH:(h + 1) * BH],
                                          acc[h][:])

    nc.finalize()
    return nc


def _get_program(key):
    key = tuple(np.asarray(key, np.float64).tolist())
    if key not in _BUILD_CACHE:
        _BUILD_CACHE[key] = _build(key)
    return _BUILD_CACHE[key]


def _spectral_prep(X8f, R, coeffs, tm, th):
    N_ = X8f.shape[1]

    def zmv(v):
        return (X8f.T @ (X8f @ v)) * ZDESCALE

    rng = np.random.default_rng(1)
    v = rng.standard_normal(N_).astype(np.float32)
    v /= np.linalg.norm(v)
    lam1 = 0.0
    for _ in range(12):
        w_ = zmv(v)
        lam1 = float(np.linalg.norm(w_))
        v = w_ / lam1
    u = v.astype(np.float64)
    u /= np.linalg.norm(u)

    v2 = rng.standard_normal(N_).astype(np.float32)
    v2 -= (u @ v2).astype(np.float32) * u.astype(np.float32)
    v2 /= np.linalg.norm(v2)
    lam2 = 0.0
    for _ in range(12):
        w_ = zmv(v2)
        w_ -= (u @ w_).astype(np.float32) * u.astype(np.float32)
        lam2 = float(np.linalg.norm(w_))
        v2 = w_ / lam2

    co = np.asarray(coeffs, np.float64)
    DEG0 = len(co) - 1

    def p_eval(x):
        x = np.asarray(x, np.float64)
        t0 = np.ones_like(x)
        t1 = x
        s = co[0] * t0 + co[1] * t1
        for k in range(2, DEG0 + 1):
            t0, t1 = t1, 2 * x * t1 - t0
            s += co[k] * t1
        return s

    s1 = (lam1 - tm) / th
    nu = (0.0 - tm) / th
    SAFETY = 1.35
    hi = (lam2 * SAFETY - tm) / th
    lo = nu
    split_ok = (lam2 * SAFETY < 0.6 * lam1) and hi > lo
    if not split_ok:
        lo = nu
        hi = (lam1 * 1.01 - tm) / th
        s1 = hi

    def cheb_fit(lo_, hi_, d_):
        j = np.arange(d_ + 1)
        theta = np.pi * (j + 0.5) / (d_ + 1)
        xn = (lo_ + hi_) / 2 + (hi_ - lo_) / 2 * np.cos(theta)
        fn = p_eval(xn)
        q_ = np.array([2.0 / (d_ + 1) * np.sum(fn * np.cos(k * theta))
                       for k in range(d_ + 1)])
        q_[0] /= 2
        return q_

    xs = np.linspace(lo, hi, 2001)
    deg = DEG0
    for d_ in range(3, DEG0 + 1):
        q = cheb_fit(lo, hi, d_)
        y = (xs - (lo + hi) / 2) / ((hi - lo) / 2)
        t0 = np.ones_like(y)
        t1 = y
        sfit = q[0] * t0 + q[1] * t1
        for k in range(2, d_ + 1):
            t0, t1 = t1, 2 * y * t1 - t0
            sfit += q[k] * t1
        if np.abs(sfit - p_eval(xs)).max() < 1.5e-4:
            deg = d_
            break
    q = cheb_fit(lo, hi, deg)

    mhat = (lo + hi) / 2
    hhat = (hi - lo) / 2
    tmp = tm + th * mhat
    thp = th * hhat

    ynu = (nu - mhat) / hhat
    t0, t1 = 1.0, ynu
    qnu = q[0] + q[1] * t1
    for k in range(2, deg + 1):
        t0, t1 = t1, 2 * ynu * t1 - t0
        qnu += q[k] * t1
    beta = p_eval(s1) - qnu if split_ok else 0.0

    w = u @ R.T.astype(np.float64)
    key = (tmp, thp, float(deg)) + tuple(q.tolist())
    return key, lam1, u, beta * w, split_ok


def _rhs_chunked(Xpart):
    """[K, 512] -> [cc, p, kk, cb]: 4KB contiguous per (cc, p)."""
    kt = Xpart.shape[0] // 128
    return np.ascontiguousarray(
        Xpart.reshape(kt // RCH, RCH, 128, CB).transpose(0, 2, 1, 3)
    ).reshape(128 * kt, CB)


def _rhs_layout(Xpart):
    kt = Xpart.shape[0] // 128
    return np.ascontiguousarray(
        Xpart.reshape(kt, 128, CB).transpose(1, 0, 2)
    ).reshape(128 * kt, CB)


def _run(X, R, coeffs, t_mid, t_half, trace=False):
    import ml_dtypes
    from concourse.bass_utils import run_bass_kernel_spmd

    X = np.ascontiguousarray(np.asarray(X, np.float32))
    R = np.ascontiguousarray(np.asarray(R, np.float32))
    coeffs = np.asarray(coeffs, np.float32)
    tm = float(np.asarray(t_mid).reshape(-1)[0])
    th = float(np.asarray(t_half).reshape(-1)[0])

    f8np = ml_dtypes.float8_e4m3
    X8 = (X * XSCALE).astype(f8np)
    X8f = X8.astype(np.float32)
    key, lam1, u, bw, split_ok = _spectral_prep(X8f, R, coeffs, tm, th)

    nc = _get_program(key)

    V32 = np.ascontiguousarray(R.T.astype(np.float32))
    V8 = V32.astype(f8np)
    VL8 = np.ascontiguousarray(
        V8.reshape(KT2, 128, NH, BH).transpose(2, 1, 0, 3)
    ).reshape(NH * 128 * KT2, BH)

    u32 = u.astype(np.float32)
    lam1_eff = lam1 if split_ok else 0.0
    BW16 = np.ascontiguousarray(bw.astype(np.float16)[None, :])

    in_maps = []
    for i in range(NC):
        ib = slice(i * CB, (i + 1) * CB)
        ui = u32[ib]
        ou_blocks = []
        for r in (1, 2, 3):
            jb = slice(((i + r) % NC) * CB, ((i + r) % NC) * CB + CB)
            ou_blocks.append(lam1_eff * np.outer(ui, u32[jb]))
        j4 = slice(((i + 4) % NC) * CB, ((i + 4) % NC) * CB + CB)
        ou_blocks.append(0.5 * lam1_eff * np.outer(ui, u32[j4]))
        ou_blocks.append(lam1_eff * np.outer(ui, ui))
        OU16 = np.concatenate(ou_blocks, axis=0).astype(np.float16)

        cflags = np.zeros((1, 80), np.int32)
        cflags[0, 0 + i] = 1                              # diag place
        for r in (1, 2, 3):
            cflags[0, 8 * r + (i + r) % 8] = 1            # self/send r
            cflags[0, 24 + 8 * r + (i - r) % 8] = 1       # recv r
        cflags[0, 56 + (i + 4) % 8] = 1                   # b4 all roles

        kh = slice(0, M // 2) if i < 4 else slice(M // 2, M)
        XSr = [X8[:, ((i + r) % NC) * CB:((i + r) % NC) * CB + CB]
               for r in (1, 2, 3)]
        in_maps.append({
            "XR8": _rhs_layout(X8[:, ib]),
            "XS8": np.concatenate([_rhs_chunked(x) for x in XSr], axis=0),
            "XS84": _rhs_chunked(X8[kh, j4]),
            "XLH8": _rhs_layout(X8[kh, ib]),
            "OU16": OU16,
            "CND32": cflags,
            "VL8": VL8,
            "Vblk32": np.ascontiguousarray(V32[ib, :]),
            "U16": np.ascontiguousarray(ui[None, :]).astype(np.float16),
            "BW16": BW16,
        })

    res = run_bass_kernel_spmd(nc, in_maps, core_ids=list(range(NC)),
                               trace=trace)

    out = np.empty((B, N), np.float32)
    for i in range(NC):
        out[:, i * CB:(i + 1) * CB] = res.results[i]["acc_out"].T
    return out, res


def kernel(X, R, coeffs, t_mid, t_half):
    out, _ = _run(X, R, coeffs, t_mid, t_half, trace=False)
    return out
